# revision 1
# baseline (speedup 1.0000x reference)
"""Trainium2 Bass kernel for nn_CombinedLoss (L1 wave + L1 on real-morlet CWT).

Math: loss = 0.5*mean|o-t| + 0.5*mean|CWT(o)-CWT(t)|.  Convolution is
linear, so CWT(o)-CWT(t) = CWT(o-t): one CWT pass over d = o-t.

Mapping: the 1-D convs (36 widths, taps 10..360) are banded-Toeplitz
matmuls on the tensor engine.  The signal is laid out transposed
(D_T[u, col] = d[128*col + u]) so the PE contracts over 128 consecutive
samples per chunk; each width needs 3 or 5 chunk matmuls (130 total).

Sharding: positions are split across the 8 cores (32768 samples each,
with 256-sample halos, zero-padded at the global edges) so every core
runs the identical SPMD program; per-core partial |.| sums are gathered
and combined on the host (the all-reduce step).
"""

import numpy as np
import ml_dtypes

import concourse.bass as bass
import concourse.tile as tile
import concourse.mybir as mybir
from concourse.bass_utils import run_bass_kernel_spmd
from concourse.masks import make_identity
from concourse.vector_clock import ScopedClock

L = 262144
NW = 36
ALPHA = 0.5
N_CORES = 8
CORE_POS = L // N_CORES          # 32768 positions per core
WIN = 49152                      # 3 chunks of 16384 (256-halo + pad)
NGROUPS = 9                      # 4 widths per reduce group
F32 = mybir.dt.float32
BF16 = mybir.dt.bfloat16
WDT = mybir.dt.bfloat16          # weight/signal dtype on the PE
WDT_NP = mybir.dt.np(WDT)


class _TC(tile.TileContext):
    """TileContext whose tail drain carries at most one sync wait.

    The walrus build in this container rejects a Drain instruction with
    more than one sync wait; emit the global-clock waits as standalone
    wait_ge instructions instead.
    """

    def _lower_ordered_insts(self, ordered):
        # Hoist all-but-one sync wait off each instruction into standalone
        # EventSemaphore waits on the same engine (in-order execution makes
        # this equivalent); walrus here allows 1 wait per instruction.
        nc = self.nc
        for bb_name in list(ordered.keys()):
            insts = ordered[bb_name]
            new = []
            for inst in insts:
                si = inst.sync_info
                if si is not None and len(si.on_wait) > 1:
                    waits = list(si.on_wait)
                    for w in waits[:-1]:
                        nop = mybir.InstEventSemaphore(
                            name=f"wsplit-{nc.next_id()}", ins=[], outs=[],
                            engine=inst.engine,
                        )
                        nop.sync_info = mybir.SyncInfo(on_wait=[w], on_update=[])
                        nc.register_instruction(nop, overwrite=True)
                        new.append(nop)
                    inst.sync_info = mybir.SyncInfo(
                        on_wait=[waits[-1]], on_update=list(si.on_update)
                    )
                new.append(inst)
            ordered[bb_name] = new
        return super()._lower_ordered_insts(ordered)

    def _drain_and_barrier(self, tick_clock, wait_clock):
        nc = self.nc
        probe = mybir.InstDrain(
            name=f"probe-{nc.next_id()}", ins=[], outs=[], engine=mybir.EngineType.SP
        )
        wait_clock.add_sem_waits(probe, ScopedClock({None: tick_clock.global_clock}))
        si = probe.sync_info
        waits = list(si.on_wait) if si is not None else []
        allocated = self.sems.allocated()
        handles = list(allocated.values()) if isinstance(allocated, dict) else list(allocated)
        id2sem = {h.num: h for h in handles}
        name2sem = {h.name: h for h in handles}
        for w in waits:
            sem = id2sem.get(w.id) or name2sem.get(w.ant_name)
            assert sem is not None, (w.id, w.ant_name, sorted(id2sem))
            nc.sync.wait_ge(sem, w.wait_value)
        nc.sync.drain()
        nc.all_engine_barrier()
        popped = nc._tile_sem_poison_stack.pop()
        assert popped is self._sem_poison
        nc.clear_and_free_semaphores(list(self.sems.allocated().values()))
        nc.all_engine_barrier()


def _morlet_flipped(N, w):
    # reference convolves with ker[::-1] of the real morlet; convolution
    # out[i] = sum_k g[k] d[i - a0 + k] uses g = that kernel re-flipped.
    x = np.linspace(-2.0 * np.pi, 2.0 * np.pi, N)
    ker = (np.cos(w * x) - np.exp(-0.5 * w * w)) * np.exp(-0.5 * x * x) * np.pi ** (-0.25)
    return ker  # ker[::-1][::-1]


def _width_meta(w):
    N = 10 * w
    a0 = 5 * w
    q = -(-a0 // 128)
    nch = (127 + (N - 1) - a0 + 128 * q) // 128 + 1
    return N, a0, q, nch


def _build_weights():
    """[128, 130*128] bf16 Toeplitz chunks, widths 1..36 in order, plus
    per-width (q, nch, tile_offset)."""
    mats = []
    meta = []
    off = 0
    for w in range(1, NW + 1):
        N, a0, q, nch = _width_meta(w)
        g = _morlet_flipped(N, float(w))
        up = np.arange(128)[:, None]
        j = np.arange(128)[None, :]
        for cc in range(nch):
            k = 128 * cc + up - j + a0 - 128 * q
            M = np.where((k >= 0) & (k < N), g[np.clip(k, 0, N - 1)], 0.0)
            mats.append(M)
        meta.append((q, nch, off))
        off += nch
    T = np.concatenate(mats, axis=1)  # [128, 130*128]
    return T.astype(WDT_NP), meta


_T_WEIGHTS, _W_META = _build_weights()
_N_TILES = _T_WEIGHTS.shape[1] // 128  # 130

# group g covers widths 4g+1 .. 4g+4
_GROUPS = []
for g in range(NGROUPS):
    ws = list(range(4 * g + 1, 4 * g + 5))
    ch0 = _W_META[ws[0] - 1][2]
    nch_g = sum(_W_META[w - 1][1] for w in ws)
    _GROUPS.append((ws, ch0, nch_g))

_NC_CACHE = None


def _build_nc():
    nc = bass.Bass("TRN2", target_bir_lowering=False, debug=False, num_devices=N_CORES)
    o_ext = nc.dram_tensor("o_win", [128, 384], F32, kind="ExternalInput")
    t_ext = nc.dram_tensor("t_win", [128, 384], F32, kind="ExternalInput")
    tw_ext = nc.dram_tensor("tw", [128, _N_TILES * 128], WDT, kind="ExternalInput")
    out_ext = nc.dram_tensor("partials", [128, 16], F32, kind="ExternalOutput")

    with _TC(nc) as tc:
        with (
            tc.tile_pool(name="const", bufs=1) as const_pool,
            tc.tile_pool(name="sig", bufs=1) as sig_pool,
            tc.tile_pool(name="dnat", bufs=1) as dnat_pool,
            tc.tile_pool(name="dt", bufs=1) as dt_pool,
            tc.tile_pool(name="wslab", bufs=1) as wslab_pool,
            tc.tile_pool(name="scratch", bufs=2) as scratch_pool,
            tc.tile_pool(name="parts", bufs=1) as parts_pool,
            tc.tile_pool(name="psd", bufs=1, space="PSUM") as psd_pool,
            tc.tile_pool(name="psc", bufs=3, space="PSUM") as psc_pool,
        ):
            ident = const_pool.tile([128, 128], BF16, tag="ident")
            make_identity(nc, ident[:])

            # PE warm-up: ~3us of dummy transposes (no data deps) so the
            # p-state/HAM ramp runs while the input DMAs are in flight.
            warm_ps = psd_pool.tile([128, 128], BF16, tag="warm")
            for _ in range(26):
                nc.tensor.transpose(warm_ps[:], ident[:], ident[:])

            # weight slabs: 3 big DMAs (3 reduce-groups each) for
            # descriptor efficiency while still overlapping with PE
            slab_tiles = []
            slab_of_group = {}
            col_in_slab = {}
            for s in range(3):
                gs = _GROUPS[3 * s:3 * s + 3]
                ch0 = gs[0][1]
                nch_s = sum(g[2] for g in gs)
                t = wslab_pool.tile([128, nch_s * 128], WDT, tag=f"w{s}")
                nc.sync.dma_start(t[:], tw_ext[:, ch0 * 128:(ch0 + nch_s) * 128])
                slab_tiles.append(t)
                for gi, (ws_, gch0, gnch) in enumerate(gs):
                    slab_of_group[3 * s + gi] = t
                    col_in_slab[3 * s + gi] = gch0 - ch0

            o_sb = sig_pool.tile([128, 384], F32, tag="o")
            nc.sync.dma_start(o_sb[:], o_ext[:])
            t_sb = sig_pool.tile([128, 384], F32, tag="t")
            nc.sync.dma_start(t_sb[:], t_ext[:])

            d_nat = dnat_pool.tile([128, 384], BF16)
            nc.vector.tensor_sub(d_nat[:], o_sb[:], t_sb[:])

            psum_d = psd_pool.tile([128, 384], BF16)
            for c in range(3):
                nc.tensor.transpose(
                    psum_d[:, 128 * c:128 * (c + 1)],
                    d_nat[:, 128 * c:128 * (c + 1)],
                    ident[:],
                )
            dt = dt_pool.tile([128, 384], WDT)
            nc.vector.tensor_copy(dt[:], psum_d[:])

            parts = parts_pool.tile([128, 16], F32)
            # wave term: own positions are D_T columns 2..258 (bf16 psum)
            nc.vector.tensor_reduce(
                parts[:, 0:1], psum_d[:, 2:258], axis=mybir.AxisListType.X,
                op=mybir.AluOpType.add, apply_absolute_value=True,
            )

            for g, (ws, ch0, nch_g) in enumerate(_GROUPS):
                psum = psc_pool.tile([128, 1024], F32, tag="conv")
                wsl = slab_of_group[g]
                toff = col_in_slab[g]
                for k, w in enumerate(ws):
                    q, nch, _ = _W_META[w - 1]
                    for cc in range(nch):
                        c0 = 2 - q + cc
                        nc.tensor.matmul(
                            psum[:, 256 * k:256 * (k + 1)],
                            wsl[:, 128 * toff:128 * (toff + 1)],
                            dt[:, c0:c0 + 256],
                            start=(cc == 0),
                            stop=(cc == nch - 1),
                        )
                        toff += 1
                if g % 2 == 0:
                    nc.vector.tensor_reduce(
                        parts[:, 1 + g:2 + g], psum[:], axis=mybir.AxisListType.X,
                        op=mybir.AluOpType.add, apply_absolute_value=True,
                    )
                else:
                    sc = scratch_pool.tile([128, 1024], F32, tag="absout")
                    nc.scalar.activation(
                        sc[:], psum[:], mybir.ActivationFunctionType.Abs,
                        accum_out=parts[:, 1 + g:2 + g],
                    )

            nc.gpsimd.dma_start(out_ext[:], parts[:])
    return nc


def _get_nc():
    global _NC_CACHE
    if _NC_CACHE is None:
        _NC_CACHE = _build_nc()
    return _NC_CACHE


def kernel(outputs, targets):
    o = np.asarray(outputs, dtype=np.float32).reshape(-1)
    t = np.asarray(targets, dtype=np.float32).reshape(-1)
    assert o.shape == (L,) and t.shape == (L,)

    in_maps = []
    for core in range(N_CORES):
        win_start = core * CORE_POS - 256
        lo, hi = max(0, win_start), min(L, win_start + WIN)
        o_win = np.zeros(WIN, np.float32)
        t_win = np.zeros(WIN, np.float32)
        o_win[lo - win_start:hi - win_start] = o[lo:hi]
        t_win[lo - win_start:hi - win_start] = t[lo:hi]
        # tile[p, 128c+q] = win[c*16384 + 128p + q]
        o_tile = o_win.reshape(3, 128, 128).transpose(1, 0, 2).reshape(128, 384)
        t_tile = t_win.reshape(3, 128, 128).transpose(1, 0, 2).reshape(128, 384)
        in_maps.append({
            "o_win": np.ascontiguousarray(o_tile),
            "t_win": np.ascontiguousarray(t_tile),
            "tw": _T_WEIGHTS,
        })

    nc = _get_nc()
    res = run_bass_kernel_spmd(nc, in_maps, core_ids=list(range(N_CORES)))

    wave = 0.0
    cwt = 0.0
    for core in range(N_CORES):
        p = np.asarray(res.results[core]["partials"], dtype=np.float64)
        wave += p[:, 0].sum()
        cwt += p[:, 1:1 + NGROUPS].sum()
    loss = ALPHA * wave / L + (1.0 - ALPHA) * cwt / (NW * L)
    return np.float32(loss)



# revision 17
# speedup vs baseline: 1.8527x; 1.8527x over previous
"""Trainium2 Bass kernel for nn_CombinedLoss (L1 wave + L1 on real-morlet CWT).

Math: loss = 0.5*mean|o-t| + 0.5*mean|CWT(o)-CWT(t)|.  Convolution is
linear, so CWT(o)-CWT(t) = CWT(o-t): one CWT pass over d = o-t.

Mapping: width-sharded SPMD (the 36 wavelet widths are distributed over
the 8 cores; every core convolves the full replicated signal with its
4-5 widths).  Each width's banded-Toeplitz conv runs on the tensor
engine as fp8 DoubleRow matmuls: one "unit" contracts 256 consecutive
input samples (2 k-tiles of 128) against a [128,2,128] weight block,
with per-width output shifts S_w = (5w mod 128) chosen so small widths
(1..12) need 1 unit and large widths (13..36) need 2 - 8 units/core.

The moving operand is an overlapping-stride AP view of the transposed
signal (host pre-transposes; tile[u, PAD+c] = d[128c+u]).  |.|-sums of
the psum banks are spread across DVE / ACT / Pool; per-core partials
are combined on the host (the all-reduce step).
"""

import numpy as np
import ml_dtypes

import concourse.bass as bass
import concourse.tile as tile
import concourse.mybir as mybir
from concourse.bass_utils import run_bass_kernel_spmd
from concourse.vector_clock import ScopedClock

L = 262144
NW = 36
ALPHA = 0.5
N_CORES = 8
NCOLS = 2048                 # output columns (128 samples each)
PAD = 4                      # zero columns left of the signal
SIGW = 2056                  # PAD + 2048 + 4
F32 = mybir.dt.float32
BF16 = mybir.dt.bfloat16
FP8 = mybir.dt.float8e4
FP8_NP = ml_dtypes.float8_e4m3

# slot structure (identical on every core): (units, c0)
SLOT_UNITS = [2, 2, 2, 1, 1]
SLOT_C0 = [-1, -1, -1, 0, 0]
UNIT_OFF = [0, 2, 4, 6, 7]   # first unit index of each slot
N_UNITS = 8
N_PHASES = 4                 # 4 x 512 output columns
WARMUP_MM = 20

# signal DMA blocks and subtract blocks (tile column ranges).  The sub
# blocks are chosen so the shift-1 subtract (reads o/t one column ahead)
# stays within the DMA blocks already landed.
DMA_BLOCKS = [(0, 520), (520, 1036), (1036, 1554), (1554, 2056)]
SUB_BLOCKS = [(0, 259), (259, 519), (519, 1035), (1035, 1553), (1553, 2055)]
D2W = 2060

# per-(phase) reduce plan: list of (bank_lo, nbanks, engine)
# banks B(s,jb) = (5*jb + s) % 8; groups are address-contiguous except the
# (7,0) wrap pair which uses a negative-stride AP
REDUCE_PLAN = [
    [(0, 2, "dve"), (2, 3, "act")],
    [(5, 3, "dve"), (0, 2, "act")],
    [(2, 2, "act"), (4, 3, "dve")],
    [(7, -2, "dve"), (1, 3, "act")],
]


def core_widths(c):
    """5 width slots for core c (0 = zero/padding slot)."""
    return [13 + c, 21 + c, 29 + c, 1 + c, 9 + c if c < 4 else 0]


def _morlet(N, w):
    x = np.linspace(-2.0 * np.pi, 2.0 * np.pi, N)
    return (np.cos(w * x) - np.exp(-0.5 * w * w)) * np.exp(-0.5 * x * x) * np.pi ** (-0.25)


def _build_core_weights(c):
    """[128, 8*256] fp8 weight layout for core c."""
    W = np.zeros((128, N_UNITS, 2, 128), np.float32)
    k = np.arange(128)[:, None]
    i = np.arange(128)[None, :]
    for s, w in enumerate(core_widths(c)):
        if w == 0:
            continue
        N, a0 = 10 * w, 5 * w
        S = a0 % 128
        g = _morlet(N, float(w))
        for u in range(SLOT_UNITS[s]):
            for t in range(2):
                m = k - i - S + a0 + 128 * (SLOT_C0[s] + 2 * u + t)
                W[:, UNIT_OFF[s] + u, t, :] = np.where(
                    (m >= 0) & (m < N), g[np.clip(m, 0, N - 1)], 0.0
                )
    return W.reshape(128, N_UNITS * 256).astype(FP8_NP)


_CORE_WEIGHTS = [_build_core_weights(c) for c in range(N_CORES)]


class _TC(tile.TileContext):
    """TileContext whose tail drain carries at most one sync wait (the
    walrus build in this container rejects multi-wait Drains)."""

    def _lower_ordered_insts(self, ordered):
        nc = self.nc
        for bb_name in list(ordered.keys()):
            insts = ordered[bb_name]
            new = []
            for inst in insts:
                si = inst.sync_info
                if si is not None and len(si.on_wait) > 1:
                    waits = list(si.on_wait)
                    for w in waits[:-1]:
                        nop = mybir.InstEventSemaphore(
                            name=f"wsplit-{nc.next_id()}", ins=[], outs=[],
                            engine=inst.engine,
                        )
                        nop.sync_info = mybir.SyncInfo(on_wait=[w], on_update=[])
                        nc.register_instruction(nop, overwrite=True)
                        new.append(nop)
                    inst.sync_info = mybir.SyncInfo(
                        on_wait=[waits[-1]], on_update=list(si.on_update)
                    )
                new.append(inst)
            ordered[bb_name] = new
        return super()._lower_ordered_insts(ordered)

    def _drain_and_barrier(self, tick_clock, wait_clock):
        nc = self.nc
        probe = mybir.InstDrain(
            name=f"probe-{nc.next_id()}", ins=[], outs=[], engine=mybir.EngineType.SP
        )
        wait_clock.add_sem_waits(probe, ScopedClock({None: tick_clock.global_clock}))
        si = probe.sync_info
        waits = list(si.on_wait) if si is not None else []
        allocated = self.sems.allocated()
        handles = list(allocated.values()) if isinstance(allocated, dict) else list(allocated)
        id2sem = {h.num: h for h in handles}
        name2sem = {h.name: h for h in handles}
        for w in waits:
            sem = id2sem.get(w.id) or name2sem.get(w.ant_name)
            assert sem is not None, (w.id, w.ant_name, sorted(id2sem))
            nc.sync.wait_ge(sem, w.wait_value)
        nc.sync.drain()
        nc.all_engine_barrier()
        popped = nc._tile_sem_poison_stack.pop()
        assert popped is self._sem_poison
        nc.clear_and_free_semaphores(list(self.sems.allocated().values()))
        nc.all_engine_barrier()


def _dr_rhs(d2_tile, base):
    """DoubleRow moving-operand view [128, 2, 512]: k-tile t of column j
    is d[128*(base+t+j)+k], served from the shift-t copy of the signal."""
    return d2_tile[:, :, base:base + 512]


_NC_CACHE = None


def _build_nc():
    nc = bass.Bass("TRN2", target_bir_lowering=False, debug=False, num_devices=N_CORES)
    o_ext = nc.dram_tensor("o_sig", [128, SIGW], FP8, kind="ExternalInput")
    t_ext = nc.dram_tensor("t_sig", [128, SIGW], FP8, kind="ExternalInput")
    w_ext = nc.dram_tensor("wts", [128, N_UNITS * 256], FP8, kind="ExternalInput")
    wave_ext = nc.dram_tensor("wave_in", [128, 512], BF16, kind="ExternalInput")
    out_ext = nc.dram_tensor("partials", [128, 16], F32, kind="ExternalOutput")

    with _TC(nc) as tc:
        with (
            tc.tile_pool(name="sig", bufs=1) as sig_pool,
            tc.tile_pool(name="wt", bufs=1) as wt_pool,
            tc.tile_pool(name="misc", bufs=1) as misc_pool,
            tc.tile_pool(name="ps", bufs=1, space="PSUM") as ps_pool,
        ):
            o_sb = sig_pool.tile([128, SIGW], FP8, tag="o")
            t_sb = sig_pool.tile([128, SIGW], FP8, tag="t")
            d2_sb = sig_pool.tile([128, 2, D2W], FP8, tag="d2")
            w_sb = wt_pool.tile([128, N_UNITS, 2, 128], FP8, tag="w")
            wave_sb = misc_pool.tile([128, 512], BF16, tag="wave")
            wd_sb = misc_pool.tile([128, 256], BF16, tag="wd")
            scratch = misc_pool.tile([128, 1536], BF16, tag="scr")
            parts = misc_pool.tile([128, 16], F32, tag="parts")
            warm = misc_pool.tile([128, 2, 128], FP8, tag="warm")
            psum = ps_pool.tile([128, 4096], F32, tag="psum")

            # ---- input DMAs (SP engine queue, in priority order) ----
            nc.sync.dma_start(o_sb[:, 0:520], o_ext[:, 0:520])
            nc.sync.dma_start(t_sb[:, 0:520], t_ext[:, 0:520])
            nc.sync.dma_start(wave_sb[:], wave_ext[:])
            # slot-0 weights (units 0,1)
            nc.sync.dma_start(w_sb[:, 0:2], w_ext[:, 0:512])
            nc.sync.dma_start(o_sb[:, 520:1036], o_ext[:, 520:1036])
            nc.sync.dma_start(t_sb[:, 520:1036], t_ext[:, 520:1036])
            nc.sync.dma_start(w_sb[:, 2:8], w_ext[:, 512:2048])
            nc.sync.dma_start(o_sb[:, 1036:1554], o_ext[:, 1036:1554])
            nc.sync.dma_start(t_sb[:, 1036:1554], t_ext[:, 1036:1554])
            nc.sync.dma_start(o_sb[:, 1554:2056], o_ext[:, 1554:2056])
            nc.sync.dma_start(t_sb[:, 1554:2056], t_ext[:, 1554:2056])

            # ---- memsets; shift-0 subs on DVE, shift-1 subs + wave on Pool
            nc.vector.memset(warm[:], 0.0)
            nc.gpsimd.memset(parts[:], 0.0)
            for bi, (lo, hi) in enumerate(SUB_BLOCKS):
                nc.vector.tensor_sub(
                    d2_sb[:, 0, lo:hi], o_sb[:, lo:hi], t_sb[:, lo:hi])
                nc.gpsimd.tensor_sub(
                    d2_sb[:, 1, lo:hi], o_sb[:, lo + 1:hi + 1], t_sb[:, lo + 1:hi + 1])
                if bi == 0:
                    nc.gpsimd.tensor_sub(
                        wd_sb[:], wave_sb[:, 0:256], wave_sb[:, 256:512])
                    nc.vector.tensor_reduce(
                        parts[:, 0:1], wd_sb[:], axis=mybir.AxisListType.X,
                        op=mybir.AluOpType.add, apply_absolute_value=True,
                    )

            # ---- PE warmup (p-state ramp bridge; garbage values, never read)
            for _ in range(WARMUP_MM):
                nc.tensor.matmul(
                    psum[:, 3584:3712], warm[:], warm[:],
                    start=True, stop=True,
                    perf_mode=mybir.MatmulPerfMode.DoubleRow,
                )

            # ---- conv matmuls + reduces, phase by phase ----
            red_col = 1
            for jb in range(N_PHASES):
                for s in range(5):
                    bank = (5 * jb + s) % 8
                    nu = SLOT_UNITS[s]
                    for u in range(nu):
                        base = PAD + SLOT_C0[s] + 2 * u + 512 * jb
                        nc.tensor.matmul(
                            psum[:, 512 * bank:512 * (bank + 1)],
                            w_sb[:, UNIT_OFF[s] + u],
                            _dr_rhs(d2_sb, base),
                            start=(u == 0), stop=(u == nu - 1),
                            perf_mode=mybir.MatmulPerfMode.DoubleRow,
                        )
                for lo_bank, nb, eng in REDUCE_PLAN[jb]:
                    if nb > 0:
                        src = psum[:, 512 * lo_bank:512 * (lo_bank + nb)]
                        axis = mybir.AxisListType.X
                    else:
                        # wrap pair (banks lo_bank and 0): negative stride
                        src = psum[:, 512 * lo_bank:512 * (lo_bank + 1)].copy()
                        src = src.unsqueeze(1)
                        src.ap[1] = [-512 * lo_bank, 2]
                        nb = -nb
                        axis = mybir.AxisListType.XY
                    if eng == "dve":
                        nc.vector.tensor_reduce(
                            parts[:, red_col:red_col + 1], src,
                            axis=axis, op=mybir.AluOpType.add,
                            apply_absolute_value=True,
                        )
                    else:
                        nc.scalar.activation(
                            scratch[:, 0:512 * nb], src,
                            mybir.ActivationFunctionType.Abs,
                            accum_out=parts[:, red_col:red_col + 1],
                        )
                    red_col += 1

            nc.gpsimd.dma_start(out_ext[:], parts[:])
    return nc


def _get_nc():
    global _NC_CACHE
    if _NC_CACHE is None:
        _NC_CACHE = _build_nc()
    return _NC_CACHE


def _sig_tile(x8):
    """[128, SIGW] fp8 transposed layout: tile[u, PAD+c] = x[128c+u]."""
    tile_ = np.zeros((128, SIGW), FP8_NP)
    tile_[:, PAD:PAD + NCOLS] = x8.reshape(NCOLS, 128).T
    return np.ascontiguousarray(tile_)


def _make_in_maps(o, t):
    o8 = _sig_tile(o.astype(FP8_NP))
    t8 = _sig_tile(t.astype(FP8_NP))
    obf = o.astype(ml_dtypes.bfloat16).reshape(NCOLS, 128).T
    tbf = t.astype(ml_dtypes.bfloat16).reshape(NCOLS, 128).T

    in_maps = []
    for c in range(N_CORES):
        wave = np.empty((128, 512), ml_dtypes.bfloat16)
        wave[:, 0:256] = obf[:, 256 * c:256 * (c + 1)]
        wave[:, 256:512] = tbf[:, 256 * c:256 * (c + 1)]
        in_maps.append({
            "o_sig": o8,
            "t_sig": t8,
            "wts": _CORE_WEIGHTS[c],
            "wave_in": np.ascontiguousarray(wave),
        })
    return in_maps


def kernel(outputs, targets):
    o = np.asarray(outputs, dtype=np.float32).reshape(-1)
    t = np.asarray(targets, dtype=np.float32).reshape(-1)
    assert o.shape == (L,) and t.shape == (L,)

    in_maps = _make_in_maps(o, t)
    nc = _get_nc()
    res = run_bass_kernel_spmd(nc, in_maps, core_ids=list(range(N_CORES)))

    wave_sum = 0.0
    cwt_sum = 0.0
    for c in range(N_CORES):
        p = np.asarray(res.results[c]["partials"], dtype=np.float64)
        wave_sum += p[:, 0].sum()
        cwt_sum += p[:, 1:13].sum()
    loss = ALPHA * wave_sum / L + (1.0 - ALPHA) * cwt_sum / (NW * L)
    return np.float32(loss)


# revision 19
# speedup vs baseline: 2.1780x; 1.1756x over previous
"""Trainium2 Bass kernel for nn_CombinedLoss (L1 wave + L1 on real-morlet CWT).

Math: loss = 0.5*mean|o-t| + 0.5*mean|CWT(o)-CWT(t)|.  Convolution is
linear, so CWT(o)-CWT(t) = CWT(d) with d = o-t.

Mapping: width-sharded SPMD (the 36 wavelet widths are distributed over
the 8 cores; every core convolves the full replicated signal with its
4-5 widths).  Each width's banded-Toeplitz conv runs on the tensor
engine as fp8 DoubleRow matmuls: one "unit" contracts 256 consecutive
input samples (2 k-tiles of 128) against a [128,2,128] weight block,
with per-width output shifts S_w = (5w mod 128) chosen so small widths
(1..12) need 1 unit and large widths (13..36) need 2 - 8 units/core.

The moving operand needs k-tile t of output column j to read signal
column (base+t+j); since the PE rejects overlapping-stride APs, the
host supplies the transposed difference signal twice (shift-0/shift-1
planes) so the DoubleRow view is a plain slice.  |.|-sums of the psum
banks are split between DVE (tensor_reduce) and ACT (activation Abs +
accum); per-core partials are combined on the host (the all-reduce).
"""

import numpy as np
import ml_dtypes

import concourse.bass as bass
import concourse.tile as tile
import concourse.mybir as mybir
from concourse.bass_utils import run_bass_kernel_spmd
from concourse.vector_clock import ScopedClock

L = 262144
NW = 36
ALPHA = 0.5
N_CORES = 8
NCOLS = 2048                 # output columns (128 samples each)
PAD = 4                      # zero columns left of the signal
SIGW = 2060                  # PAD + 2048 + 8
WAVE0 = SIGW                 # wave-slice columns start
D2W = SIGW + 256             # + per-core wave slice
F32 = mybir.dt.float32
BF16 = mybir.dt.bfloat16
FP8 = mybir.dt.float8e4
FP8_NP = ml_dtypes.float8_e4m3

# slot structure (identical on every core): (units, c0)
SLOT_UNITS = [2, 2, 2, 1, 1]
SLOT_C0 = [-1, -1, -1, 0, 0]
UNIT_OFF = [0, 2, 4, 6, 7]   # first unit index of each slot
N_UNITS = 8
N_PHASES = 4                 # 4 x 512 output columns
WARMUP_MM = 20

# input DMA blocks (d2 plane-pair column ranges)
D2_BLOCKS = [(0, 520), (520, 1554), (1554, D2W)]

# per-phase reduce plan: (bank_lo, nbanks, engine); banks B(s,jb)=(5jb+s)%8.
# nbanks=-2 marks the (7,0) wrap pair (negative-stride AP).
REDUCE_PLAN = [
    [(0, 2, "dve"), (2, 3, "act")],
    [(5, 3, "act"), (0, 2, "dve")],
    [(2, 2, "dve"), (4, 3, "act")],
    [(7, -2, "act"), (1, 3, "dve")],
]


def core_widths(c):
    """5 width slots for core c (0 = zero/padding slot)."""
    return [13 + c, 21 + c, 29 + c, 1 + c, 9 + c if c < 4 else 0]


def _morlet(N, w):
    x = np.linspace(-2.0 * np.pi, 2.0 * np.pi, N)
    return (np.cos(w * x) - np.exp(-0.5 * w * w)) * np.exp(-0.5 * x * x) * np.pi ** (-0.25)


def _build_core_weights(c):
    """[128, 8*256] fp8 weight layout for core c."""
    W = np.zeros((128, N_UNITS, 2, 128), np.float32)
    k = np.arange(128)[:, None]
    i = np.arange(128)[None, :]
    for s, w in enumerate(core_widths(c)):
        if w == 0:
            continue
        N, a0 = 10 * w, 5 * w
        S = a0 % 128
        g = _morlet(N, float(w))
        for u in range(SLOT_UNITS[s]):
            for t in range(2):
                m = k - i - S + a0 + 128 * (SLOT_C0[s] + 2 * u + t)
                W[:, UNIT_OFF[s] + u, t, :] = np.where(
                    (m >= 0) & (m < N), g[np.clip(m, 0, N - 1)], 0.0
                )
    return W.reshape(128, N_UNITS * 256).astype(FP8_NP)


_CORE_WEIGHTS = [_build_core_weights(c) for c in range(N_CORES)]


class _TC(tile.TileContext):
    """TileContext whose tail drain carries at most one sync wait (the
    walrus build in this container rejects multi-wait Drains)."""

    def _lower_ordered_insts(self, ordered):
        nc = self.nc
        for bb_name in list(ordered.keys()):
            insts = ordered[bb_name]
            new = []
            for inst in insts:
                si = inst.sync_info
                if si is not None and len(si.on_wait) > 1:
                    waits = list(si.on_wait)
                    for w in waits[:-1]:
                        nop = mybir.InstEventSemaphore(
                            name=f"wsplit-{nc.next_id()}", ins=[], outs=[],
                            engine=inst.engine,
                        )
                        nop.sync_info = mybir.SyncInfo(on_wait=[w], on_update=[])
                        nc.register_instruction(nop, overwrite=True)
                        new.append(nop)
                    inst.sync_info = mybir.SyncInfo(
                        on_wait=[waits[-1]], on_update=list(si.on_update)
                    )
                new.append(inst)
            ordered[bb_name] = new
        return super()._lower_ordered_insts(ordered)

    def _drain_and_barrier(self, tick_clock, wait_clock):
        nc = self.nc
        probe = mybir.InstDrain(
            name=f"probe-{nc.next_id()}", ins=[], outs=[], engine=mybir.EngineType.SP
        )
        wait_clock.add_sem_waits(probe, ScopedClock({None: tick_clock.global_clock}))
        si = probe.sync_info
        waits = list(si.on_wait) if si is not None else []
        allocated = self.sems.allocated()
        handles = list(allocated.values()) if isinstance(allocated, dict) else list(allocated)
        id2sem = {h.num: h for h in handles}
        name2sem = {h.name: h for h in handles}
        for w in waits:
            sem = id2sem.get(w.id) or name2sem.get(w.ant_name)
            assert sem is not None, (w.id, w.ant_name, sorted(id2sem))
            nc.sync.wait_ge(sem, w.wait_value)
        nc.sync.drain()
        nc.all_engine_barrier()
        popped = nc._tile_sem_poison_stack.pop()
        assert popped is self._sem_poison
        nc.clear_and_free_semaphores(list(self.sems.allocated().values()))
        nc.all_engine_barrier()


_NC_CACHE = None


def _build_nc():
    nc = bass.Bass("TRN2", target_bir_lowering=False, debug=False, num_devices=N_CORES)
    d2_ext = nc.dram_tensor("d2", [128, 2, D2W], FP8, kind="ExternalInput")
    w_ext = nc.dram_tensor("wts", [128, N_UNITS * 256], FP8, kind="ExternalInput")
    out_ext = nc.dram_tensor("partials", [128, 16], F32, kind="ExternalOutput")

    with _TC(nc) as tc:
        with (
            tc.tile_pool(name="sig", bufs=1) as sig_pool,
            tc.tile_pool(name="wt", bufs=1) as wt_pool,
            tc.tile_pool(name="misc", bufs=1) as misc_pool,
            tc.tile_pool(name="ps", bufs=1, space="PSUM") as ps_pool,
        ):
            d2_sb = sig_pool.tile([128, 2, D2W], FP8, tag="d2")
            w_sb = wt_pool.tile([128, N_UNITS, 2, 128], FP8, tag="w")
            scratch = misc_pool.tile([128, 1536], BF16, tag="scr")
            parts = misc_pool.tile([128, 16], F32, tag="parts")
            warm = misc_pool.tile([128, 2, 128], FP8, tag="warm")
            psum = ps_pool.tile([128, 4096], F32, tag="psum")

            # ---- input DMAs (SP engine queue, in priority order) ----
            lo, hi = D2_BLOCKS[0]
            nc.sync.dma_start(d2_sb[:, :, lo:hi], d2_ext[:, :, lo:hi])
            nc.sync.dma_start(w_sb[:, 0:4], w_ext[:, 0:1024])      # slots 0,1
            nc.sync.dma_start(w_sb[:, 4:8], w_ext[:, 1024:2048])   # slots 2,3,4
            for lo, hi in D2_BLOCKS[1:]:
                nc.sync.dma_start(d2_sb[:, :, lo:hi], d2_ext[:, :, lo:hi])

            nc.vector.memset(warm[:], 0.0)
            nc.gpsimd.memset(parts[:], 0.0)

            # ---- wave term: |d| over this core's slice (DVE) ----
            nc.vector.tensor_reduce(
                parts[:, 0:1], d2_sb[:, 0, WAVE0:WAVE0 + 256],
                axis=mybir.AxisListType.X, op=mybir.AluOpType.add,
                apply_absolute_value=True,
            )

            # ---- PE warmup (p-state ramp bridge; zeros, never read) ----
            for _ in range(WARMUP_MM):
                nc.tensor.matmul(
                    psum[:, 3584:3712], warm[:], warm[:],
                    start=True, stop=True,
                    perf_mode=mybir.MatmulPerfMode.DoubleRow,
                )

            # ---- conv matmuls + reduces, phase by phase ----
            red_col = 1
            for jb in range(N_PHASES):
                for s in range(5):
                    bank = (5 * jb + s) % 8
                    nu = SLOT_UNITS[s]
                    for u in range(nu):
                        base = PAD + SLOT_C0[s] + 2 * u + 512 * jb
                        nc.tensor.matmul(
                            psum[:, 512 * bank:512 * (bank + 1)],
                            w_sb[:, UNIT_OFF[s] + u],
                            d2_sb[:, :, base:base + 512],
                            start=(u == 0), stop=(u == nu - 1),
                            perf_mode=mybir.MatmulPerfMode.DoubleRow,
                        )
                for lo_bank, nb, eng in REDUCE_PLAN[jb]:
                    wrap = nb < 0
                    if not wrap:
                        src = psum[:, 512 * lo_bank:512 * (lo_bank + nb)]
                        axis = mybir.AxisListType.X
                    else:
                        nb = -nb
                        src = psum[:, 512 * lo_bank:512 * (lo_bank + 1)].copy()
                        src = src.unsqueeze(1)
                        src.ap[1] = [-512 * lo_bank, 2]
                        axis = mybir.AxisListType.XY
                    if eng == "dve":
                        nc.vector.tensor_reduce(
                            parts[:, red_col:red_col + 1], src,
                            axis=axis, op=mybir.AluOpType.add,
                            apply_absolute_value=True,
                        )
                    else:
                        if wrap:
                            out_v = scratch[:, 0:512].copy().unsqueeze(1)
                            out_v.ap[1] = [512, 2]
                        else:
                            out_v = scratch[:, 0:512 * nb]
                        nc.scalar.activation(
                            out_v, src,
                            mybir.ActivationFunctionType.Abs,
                            accum_out=parts[:, red_col:red_col + 1],
                        )
                    red_col += 1

            nc.gpsimd.dma_start(out_ext[:], parts[:])
    return nc


def _get_nc():
    global _NC_CACHE
    if _NC_CACHE is None:
        _NC_CACHE = _build_nc()
    return _NC_CACHE


def _make_in_maps(o, t):
    d = (o - t).astype(FP8_NP)
    dT = d.reshape(NCOLS, 128).T                      # [128, 2048]
    d2 = np.zeros((128, 2, D2W), FP8_NP)
    d2[:, 0, PAD:PAD + NCOLS] = dT
    d2[:, 1, PAD - 1:PAD - 1 + NCOLS] = dT            # shift-1 plane

    in_maps = []
    for c in range(N_CORES):
        m = d2.copy()
        m[:, 0, WAVE0:WAVE0 + 256] = dT[:, 256 * c:256 * (c + 1)]
        in_maps.append({"d2": m, "wts": _CORE_WEIGHTS[c]})
    return in_maps


def kernel(outputs, targets):
    o = np.asarray(outputs, dtype=np.float32).reshape(-1)
    t = np.asarray(targets, dtype=np.float32).reshape(-1)
    assert o.shape == (L,) and t.shape == (L,)

    in_maps = _make_in_maps(o, t)
    nc = _get_nc()
    res = run_bass_kernel_spmd(nc, in_maps, core_ids=list(range(N_CORES)))

    wave_sum = 0.0
    cwt_sum = 0.0
    for c in range(N_CORES):
        p = np.asarray(res.results[c]["partials"], dtype=np.float64)
        wave_sum += p[:, 0].sum()
        cwt_sum += p[:, 1:9].sum()
    loss = ALPHA * wave_sum / L + (1.0 - ALPHA) * cwt_sum / (NW * L)
    return np.float32(loss)


# revision 23
# speedup vs baseline: 2.4148x; 1.1087x over previous
"""Trainium2 Bass kernel for nn_CombinedLoss (L1 wave + L1 on real-morlet CWT).

Math: loss = 0.5*mean|o-t| + 0.5*mean|CWT(o)-CWT(t)|.  Convolution is
linear, so CWT(o)-CWT(t) = CWT(d) with d = o-t.

Mapping: width-sharded SPMD (the 36 wavelet widths are distributed over
the 8 cores; every core convolves the full replicated signal with its
4-5 widths).  Each width's banded-Toeplitz conv runs on the tensor
engine as fp8 DoubleRow matmuls: one "unit" contracts 256 consecutive
input samples (2 k-tiles of 128) against a [128,2,128] weight block,
with per-width output shifts S_w = (5w mod 128) chosen so small widths
(1..12) need 1 unit and large widths (13..36) need 2 - 8 units/core.

The moving operand needs k-tile t of output column j to read signal
column (base+t+j); since the PE rejects overlapping-stride APs, the
host supplies the transposed difference signal twice (shift-0/shift-1
planes) so the DoubleRow view is a plain slice.  |.|-sums of the psum
banks are split between DVE (tensor_reduce) and ACT (activation Abs +
accum); per-core partials are combined on the host (the all-reduce).
"""

import numpy as np
import ml_dtypes

import concourse.bass as bass
import concourse.tile as tile
import concourse.mybir as mybir
from concourse.bass_utils import run_bass_kernel_spmd
from concourse.vector_clock import ScopedClock

L = 262144
NW = 36
ALPHA = 0.5
N_CORES = 8
NCOLS = 2048                 # output columns (128 samples each)
PAD = 4                      # zero columns left of the signal
SIGW = 2060                  # PAD + 2048 + 8
WAVE0 = SIGW                 # wave-slice columns start
D2W = SIGW + 256             # + per-core wave slice
F32 = mybir.dt.float32
BF16 = mybir.dt.bfloat16
FP8 = mybir.dt.float8e4
FP8_NP = ml_dtypes.float8_e4m3

# slot structure (identical on every core): (units, c0)
SLOT_UNITS = [2, 2, 2, 1, 1]
SLOT_C0 = [-1, -1, -1, 0, 0]
UNIT_OFF = [0, 2, 4, 6, 7]   # first unit index of each slot
N_UNITS = 8
N_PHASES = 4                 # 4 x 512 output columns
WARMUP_MM = 20

# input DMA blocks (d2 plane-pair column ranges)
D2_BLOCKS = [(0, 520), (520, 1554), (1554, D2W)]

# per-phase psum bank map (slot -> bank) and reduce plan (bank_lo, nbanks,
# engine).  All reduce groups are address-contiguous; triples alternate
# between ACT and DVE so neither engine's chain gates two phases in a row.
BANK_MAP = [
    [0, 1, 2, 3, 4],
    [5, 6, 7, 0, 1],
    [2, 3, 4, 5, 6],
    [5, 6, 7, 3, 4],
]
REDUCE_PLAN = [
    [(0, 2, "dve"), (2, 3, "act")],
    [(5, 3, "dve"), (0, 2, "act")],
    [(2, 3, "act"), (5, 2, "dve")],
    [(5, 3, "dve"), (3, 2, "act")],
]


def core_widths(c):
    """5 width slots for core c (0 = zero/padding slot)."""
    return [13 + c, 21 + c, 29 + c, 1 + c, 9 + c if c < 4 else 0]


def _morlet(N, w):
    x = np.linspace(-2.0 * np.pi, 2.0 * np.pi, N)
    return (np.cos(w * x) - np.exp(-0.5 * w * w)) * np.exp(-0.5 * x * x) * np.pi ** (-0.25)


def _build_core_weights(c):
    """[128, 8*256] fp8 weight layout for core c."""
    W = np.zeros((128, N_UNITS, 2, 128), np.float32)
    k = np.arange(128)[:, None]
    i = np.arange(128)[None, :]
    for s, w in enumerate(core_widths(c)):
        if w == 0:
            continue
        N, a0 = 10 * w, 5 * w
        S = a0 % 128
        g = _morlet(N, float(w))
        for u in range(SLOT_UNITS[s]):
            for t in range(2):
                m = k - i - S + a0 + 128 * (SLOT_C0[s] + 2 * u + t)
                W[:, UNIT_OFF[s] + u, t, :] = np.where(
                    (m >= 0) & (m < N), g[np.clip(m, 0, N - 1)], 0.0
                )
    return W.reshape(128, N_UNITS * 256).astype(FP8_NP)


_CORE_WEIGHTS = [_build_core_weights(c) for c in range(N_CORES)]


class _TC(tile.TileContext):
    """TileContext whose tail drain carries at most one sync wait (the
    walrus build in this container rejects multi-wait Drains)."""

    def _lower_ordered_insts(self, ordered):
        nc = self.nc
        for bb_name in list(ordered.keys()):
            insts = ordered[bb_name]
            new = []
            for inst in insts:
                si = inst.sync_info
                if si is not None and len(si.on_wait) > 1:
                    waits = list(si.on_wait)
                    for w in waits[:-1]:
                        nop = mybir.InstEventSemaphore(
                            name=f"wsplit-{nc.next_id()}", ins=[], outs=[],
                            engine=inst.engine,
                        )
                        nop.sync_info = mybir.SyncInfo(on_wait=[w], on_update=[])
                        nc.register_instruction(nop, overwrite=True)
                        new.append(nop)
                    inst.sync_info = mybir.SyncInfo(
                        on_wait=[waits[-1]], on_update=list(si.on_update)
                    )
                new.append(inst)
            ordered[bb_name] = new
        return super()._lower_ordered_insts(ordered)

    def _drain_and_barrier(self, tick_clock, wait_clock):
        nc = self.nc
        probe = mybir.InstDrain(
            name=f"probe-{nc.next_id()}", ins=[], outs=[], engine=mybir.EngineType.SP
        )
        wait_clock.add_sem_waits(probe, ScopedClock({None: tick_clock.global_clock}))
        si = probe.sync_info
        waits = list(si.on_wait) if si is not None else []
        allocated = self.sems.allocated()
        handles = list(allocated.values()) if isinstance(allocated, dict) else list(allocated)
        id2sem = {h.num: h for h in handles}
        name2sem = {h.name: h for h in handles}
        for w in waits:
            sem = id2sem.get(w.id) or name2sem.get(w.ant_name)
            assert sem is not None, (w.id, w.ant_name, sorted(id2sem))
            nc.sync.wait_ge(sem, w.wait_value)
        nc.sync.drain()
        nc.all_engine_barrier()
        popped = nc._tile_sem_poison_stack.pop()
        assert popped is self._sem_poison
        nc.clear_and_free_semaphores(list(self.sems.allocated().values()))
        nc.all_engine_barrier()


_NC_CACHE = None


def _build_nc():
    nc = bass.Bass("TRN2", target_bir_lowering=False, debug=False, num_devices=N_CORES)
    d2_ext = nc.dram_tensor("d2", [128, 2, D2W], FP8, kind="ExternalInput")
    w_ext = nc.dram_tensor("wts", [128, N_UNITS * 256], FP8, kind="ExternalInput")
    out_ext = nc.dram_tensor("partials", [128, 16], F32, kind="ExternalOutput")

    with _TC(nc) as tc:
        with (
            tc.tile_pool(name="sig", bufs=1) as sig_pool,
            tc.tile_pool(name="wt", bufs=1) as wt_pool,
            tc.tile_pool(name="misc", bufs=1) as misc_pool,
            tc.tile_pool(name="ps", bufs=1, space="PSUM") as ps_pool,
        ):
            d2_sb = sig_pool.tile([128, 2, D2W], FP8, tag="d2")
            w_sb = wt_pool.tile([128, N_UNITS, 2, 128], FP8, tag="w")
            scratch = misc_pool.tile([128, 1536], BF16, tag="scr")
            parts = misc_pool.tile([128, 16], F32, tag="parts")
            warm = misc_pool.tile([128, 2, 128], FP8, tag="warm")
            psum = ps_pool.tile([128, 4096], F32, tag="psum")

            # ---- input DMAs (SP engine queue, in priority order) ----
            lo, hi = D2_BLOCKS[0]
            nc.sync.dma_start(d2_sb[:, :, lo:hi], d2_ext[:, :, lo:hi])
            nc.sync.dma_start(w_sb[:, 0:4], w_ext[:, 0:1024])      # slots 0,1
            nc.sync.dma_start(w_sb[:, 4:8], w_ext[:, 1024:2048])   # slots 2,3,4
            for lo, hi in D2_BLOCKS[1:]:
                nc.sync.dma_start(d2_sb[:, :, lo:hi], d2_ext[:, :, lo:hi])

            nc.vector.memset(warm[:], 0.0)
            nc.gpsimd.memset(parts[:], 0.0)

            # ---- PE warmup (p-state ramp bridge; zeros, never read) ----
            for _ in range(WARMUP_MM):
                nc.tensor.matmul(
                    psum[:, 3584:3712], warm[:], warm[:],
                    start=True, stop=True,
                    perf_mode=mybir.MatmulPerfMode.DoubleRow,
                )

            # ---- conv matmuls + reduces, phase by phase ----
            red_col = 1
            for jb in range(N_PHASES):
                for s in range(5):
                    bank = BANK_MAP[jb][s]
                    nu = SLOT_UNITS[s]
                    for u in range(nu):
                        base = PAD + SLOT_C0[s] + 2 * u + 512 * jb
                        nc.tensor.matmul(
                            psum[:, 512 * bank:512 * (bank + 1)],
                            w_sb[:, UNIT_OFF[s] + u],
                            d2_sb[:, :, base:base + 512],
                            start=(u == 0), stop=(u == nu - 1),
                            perf_mode=mybir.MatmulPerfMode.DoubleRow,
                        )
                for lo_bank, nb, eng in REDUCE_PLAN[jb]:
                    src = psum[:, 512 * lo_bank:512 * (lo_bank + nb)]
                    if eng == "dve":
                        nc.vector.tensor_reduce(
                            parts[:, red_col:red_col + 1], src,
                            axis=mybir.AxisListType.X, op=mybir.AluOpType.add,
                            apply_absolute_value=True,
                        )
                    else:
                        nc.scalar.activation(
                            scratch[:, 0:512 * nb], src,
                            mybir.ActivationFunctionType.Abs,
                            accum_out=parts[:, red_col:red_col + 1],
                        )
                    red_col += 1

            # ---- wave term: |d| over this core's slice (ACT, off the
            # critical reduce chain) ----
            nc.scalar.activation(
                scratch[:, 0:256], d2_sb[:, 0, WAVE0:WAVE0 + 256],
                mybir.ActivationFunctionType.Abs,
                accum_out=parts[:, 0:1],
            )

            nc.sync.dma_start(out_ext[:], parts[:])
    return nc


def _get_nc():
    global _NC_CACHE
    if _NC_CACHE is None:
        _NC_CACHE = _build_nc()
    return _NC_CACHE


def _make_in_maps(o, t):
    d = (o - t).astype(FP8_NP)
    dT = d.reshape(NCOLS, 128).T                      # [128, 2048]
    d2 = np.zeros((128, 2, D2W), FP8_NP)
    d2[:, 0, PAD:PAD + NCOLS] = dT
    d2[:, 1, PAD - 1:PAD - 1 + NCOLS] = dT            # shift-1 plane

    in_maps = []
    for c in range(N_CORES):
        m = d2.copy()
        m[:, 0, WAVE0:WAVE0 + 256] = dT[:, 256 * c:256 * (c + 1)]
        in_maps.append({"d2": m, "wts": _CORE_WEIGHTS[c]})
    return in_maps


def kernel(outputs, targets):
    o = np.asarray(outputs, dtype=np.float32).reshape(-1)
    t = np.asarray(targets, dtype=np.float32).reshape(-1)
    assert o.shape == (L,) and t.shape == (L,)

    in_maps = _make_in_maps(o, t)
    nc = _get_nc()
    res = run_bass_kernel_spmd(nc, in_maps, core_ids=list(range(N_CORES)))

    wave_sum = 0.0
    cwt_sum = 0.0
    for c in range(N_CORES):
        p = np.asarray(res.results[c]["partials"], dtype=np.float64)
        wave_sum += p[:, 0].sum()
        cwt_sum += p[:, 1:9].sum()
    loss = ALPHA * wave_sum / L + (1.0 - ALPHA) * cwt_sum / (NW * L)
    return np.float32(loss)


# revision 26
# speedup vs baseline: 2.5208x; 1.0439x over previous
"""Trainium2 Bass kernel for nn_CombinedLoss (L1 wave + L1 on real-morlet CWT).

Math: loss = 0.5*mean|o-t| + 0.5*mean|CWT(o)-CWT(t)|.  Convolution is
linear, so CWT(o)-CWT(t) = CWT(d) with d = o-t.

Mapping: width-sharded SPMD (the 36 wavelet widths are distributed over
the 8 cores; every core convolves the full replicated signal with its
4-5 widths).  Each width's banded-Toeplitz conv runs on the tensor
engine as fp8 DoubleRow matmuls: one "unit" contracts 256 consecutive
input samples (2 k-tiles of 128) against a [128,2,128] weight block,
with per-width output shifts S_w = (5w mod 128) chosen so small widths
(1..12) need 1 unit and large widths (13..36) need 2 - 8 units/core.

The moving operand needs k-tile t of output column j to read signal
column (base+t+j); since the PE rejects overlapping-stride APs, the
host supplies the transposed difference signal twice (shift-0/shift-1
planes) so the DoubleRow view is a plain slice.  |.|-sums of the psum
banks are split between DVE (tensor_reduce) and ACT (activation Abs +
accum); per-core partials are combined on the host (the all-reduce).
"""

import numpy as np
import ml_dtypes

import concourse.bass as bass
import concourse.tile as tile
import concourse.mybir as mybir
from concourse.bass_utils import run_bass_kernel_spmd
from concourse.vector_clock import ScopedClock

L = 262144
NW = 36
ALPHA = 0.5
N_CORES = 8
NCOLS = 2048                 # output columns (128 samples each)
PAD = 4                      # zero columns left of the signal
SIGW = 2060                  # PAD + 2048 + 8
WAVE0 = SIGW                 # wave-slice columns start
D2W = SIGW + 256             # + per-core wave slice
F32 = mybir.dt.float32
BF16 = mybir.dt.bfloat16
FP8 = mybir.dt.float8e4
FP8_NP = ml_dtypes.float8_e4m3

# slot structure (identical on every core): (units, c0)
SLOT_UNITS = [2, 2, 2, 1, 1]
SLOT_C0 = [-1, -1, -1, 0, 0]
UNIT_OFF = [0, 2, 4, 6, 7]   # first unit index of each slot
N_UNITS = 8
N_PHASES = 4                 # 4 x 512 output columns
WARMUP_MM = 42

# input DMA blocks (d2 plane-pair column ranges)
D2_BLOCKS = [(0, 520), (520, 1554), (1554, D2W)]

# per-phase psum bank map (slot -> bank) and reduce plan (bank_lo, nbanks,
# engine).  All reduce groups are address-contiguous; triples alternate
# between ACT and DVE so neither engine's chain gates two phases in a row.
BANK_MAP = [
    [0, 1, 2, 3, 4],
    [5, 6, 7, 0, 1],
    [2, 3, 4, 5, 6],
    [5, 6, 7, 3, 4],
]
REDUCE_PLAN = [
    [(0, 2, "dve"), (2, 3, "act")],
    [(5, 3, "dve"), (0, 2, "act")],
    [(2, 3, "act"), (5, 2, "dve")],
    [(5, 3, "dve"), (3, 2, "act")],
]


def core_widths(c):
    """5 width slots for core c (0 = zero/padding slot)."""
    return [13 + c, 21 + c, 29 + c, 1 + c, 9 + c if c < 4 else 0]


def _morlet(N, w):
    x = np.linspace(-2.0 * np.pi, 2.0 * np.pi, N)
    return (np.cos(w * x) - np.exp(-0.5 * w * w)) * np.exp(-0.5 * x * x) * np.pi ** (-0.25)


def _build_core_weights(c):
    """[128, 8*256] fp8 weight layout for core c."""
    W = np.zeros((128, N_UNITS, 2, 128), np.float32)
    k = np.arange(128)[:, None]
    i = np.arange(128)[None, :]
    for s, w in enumerate(core_widths(c)):
        if w == 0:
            continue
        N, a0 = 10 * w, 5 * w
        S = a0 % 128
        g = _morlet(N, float(w))
        for u in range(SLOT_UNITS[s]):
            for t in range(2):
                m = k - i - S + a0 + 128 * (SLOT_C0[s] + 2 * u + t)
                W[:, UNIT_OFF[s] + u, t, :] = np.where(
                    (m >= 0) & (m < N), g[np.clip(m, 0, N - 1)], 0.0
                )
    return W.reshape(128, N_UNITS * 256).astype(FP8_NP)


_CORE_WEIGHTS = [_build_core_weights(c) for c in range(N_CORES)]


class _TC(tile.TileContext):
    """TileContext whose tail drain carries at most one sync wait (the
    walrus build in this container rejects multi-wait Drains)."""

    def _lower_ordered_insts(self, ordered):
        nc = self.nc
        for bb_name in list(ordered.keys()):
            insts = ordered[bb_name]
            new = []
            for inst in insts:
                si = inst.sync_info
                if si is not None and len(si.on_wait) > 1:
                    waits = list(si.on_wait)
                    for w in waits[:-1]:
                        nop = mybir.InstEventSemaphore(
                            name=f"wsplit-{nc.next_id()}", ins=[], outs=[],
                            engine=inst.engine,
                        )
                        nop.sync_info = mybir.SyncInfo(on_wait=[w], on_update=[])
                        nc.register_instruction(nop, overwrite=True)
                        new.append(nop)
                    inst.sync_info = mybir.SyncInfo(
                        on_wait=[waits[-1]], on_update=list(si.on_update)
                    )
                new.append(inst)
            ordered[bb_name] = new
        return super()._lower_ordered_insts(ordered)

    def _drain_and_barrier(self, tick_clock, wait_clock):
        nc = self.nc
        probe = mybir.InstDrain(
            name=f"probe-{nc.next_id()}", ins=[], outs=[], engine=mybir.EngineType.SP
        )
        wait_clock.add_sem_waits(probe, ScopedClock({None: tick_clock.global_clock}))
        si = probe.sync_info
        waits = list(si.on_wait) if si is not None else []
        allocated = self.sems.allocated()
        handles = list(allocated.values()) if isinstance(allocated, dict) else list(allocated)
        id2sem = {h.num: h for h in handles}
        name2sem = {h.name: h for h in handles}
        for w in waits:
            sem = id2sem.get(w.id) or name2sem.get(w.ant_name)
            assert sem is not None, (w.id, w.ant_name, sorted(id2sem))
            nc.sync.wait_ge(sem, w.wait_value)
        nc.sync.drain()
        nc.all_engine_barrier()
        popped = nc._tile_sem_poison_stack.pop()
        assert popped is self._sem_poison
        nc.clear_and_free_semaphores(list(self.sems.allocated().values()))
        nc.all_engine_barrier()


_NC_CACHE = None


def _build_nc():
    nc = bass.Bass("TRN2", target_bir_lowering=False, debug=False, num_devices=N_CORES)
    d2_ext = nc.dram_tensor("d2", [128, 2, D2W], FP8, kind="ExternalInput")
    w_ext = nc.dram_tensor("wts", [128, N_UNITS * 256], FP8, kind="ExternalInput")
    out_ext = nc.dram_tensor("partials", [128, 16], F32, kind="ExternalOutput")

    with _TC(nc) as tc:
        with (
            tc.tile_pool(name="sig", bufs=1) as sig_pool,
            tc.tile_pool(name="wt", bufs=1) as wt_pool,
            tc.tile_pool(name="misc", bufs=1) as misc_pool,
            tc.tile_pool(name="ps", bufs=1, space="PSUM") as ps_pool,
        ):
            d2_sb = sig_pool.tile([128, 2, D2W], FP8, tag="d2")
            w_sb = wt_pool.tile([128, N_UNITS, 2, 128], FP8, tag="w")
            scratch = misc_pool.tile([128, 1536], BF16, tag="scr")
            parts = misc_pool.tile([128, 16], F32, tag="parts")
            warm = misc_pool.tile([128, 2, 128], FP8, tag="warm")
            psum = ps_pool.tile([128, 4096], F32, tag="psum")

            # ---- input DMAs (SP engine queue, in priority order) ----
            lo, hi = D2_BLOCKS[0]
            nc.sync.dma_start(d2_sb[:, :, lo:hi], d2_ext[:, :, lo:hi])
            nc.sync.dma_start(w_sb[:, 0:4], w_ext[:, 0:1024])      # slots 0,1
            nc.sync.dma_start(w_sb[:, 4:8], w_ext[:, 1024:2048])   # slots 2,3,4
            for lo, hi in D2_BLOCKS[1:]:
                nc.sync.dma_start(d2_sb[:, :, lo:hi], d2_ext[:, :, lo:hi])

            nc.gpsimd.memset(warm[:], 0.0)
            nc.gpsimd.memset(parts[:], 0.0)

            # ---- PE warmup (p-state ramp bridge; zeros, never read) ----
            for _ in range(WARMUP_MM):
                nc.tensor.matmul(
                    psum[:, 3584:3712], warm[:], warm[:],
                    start=True, stop=True,
                    perf_mode=mybir.MatmulPerfMode.DoubleRow,
                )

            # ---- conv matmuls + reduces, phase by phase ----
            red_col = 1
            for jb in range(N_PHASES):
                for s in range(5):
                    bank = BANK_MAP[jb][s]
                    nu = SLOT_UNITS[s]
                    for u in range(nu):
                        base = PAD + SLOT_C0[s] + 2 * u + 512 * jb
                        nc.tensor.matmul(
                            psum[:, 512 * bank:512 * (bank + 1)],
                            w_sb[:, UNIT_OFF[s] + u],
                            d2_sb[:, :, base:base + 512],
                            start=(u == 0), stop=(u == nu - 1),
                            perf_mode=mybir.MatmulPerfMode.DoubleRow,
                        )
                for lo_bank, nb, eng in REDUCE_PLAN[jb]:
                    src = psum[:, 512 * lo_bank:512 * (lo_bank + nb)]
                    if eng == "dve":
                        nc.vector.tensor_reduce(
                            parts[:, red_col:red_col + 1], src,
                            axis=mybir.AxisListType.X, op=mybir.AluOpType.add,
                            apply_absolute_value=True,
                        )
                    else:
                        nc.scalar.activation(
                            scratch[:, 0:512 * nb], src,
                            mybir.ActivationFunctionType.Abs,
                            accum_out=parts[:, red_col:red_col + 1],
                        )
                    red_col += 1
                if jb == 1:
                    # wave term: |d| over this core's slice, in ACT's idle
                    # window between its ph1 and ph2 reduces
                    nc.scalar.activation(
                        scratch[:, 0:256], d2_sb[:, 0, WAVE0:WAVE0 + 256],
                        mybir.ActivationFunctionType.Abs,
                        accum_out=parts[:, 0:1],
                    )

            nc.sync.dma_start(out_ext[:], parts[:])
    return nc


def _get_nc():
    global _NC_CACHE
    if _NC_CACHE is None:
        _NC_CACHE = _build_nc()
    return _NC_CACHE


def _make_in_maps(o, t):
    d = (o - t).astype(FP8_NP)
    dT = d.reshape(NCOLS, 128).T                      # [128, 2048]
    d2 = np.zeros((128, 2, D2W), FP8_NP)
    d2[:, 0, PAD:PAD + NCOLS] = dT
    d2[:, 1, PAD - 1:PAD - 1 + NCOLS] = dT            # shift-1 plane

    in_maps = []
    for c in range(N_CORES):
        m = d2.copy()
        m[:, 0, WAVE0:WAVE0 + 256] = dT[:, 256 * c:256 * (c + 1)]
        in_maps.append({"d2": m, "wts": _CORE_WEIGHTS[c]})
    return in_maps


def kernel(outputs, targets):
    o = np.asarray(outputs, dtype=np.float32).reshape(-1)
    t = np.asarray(targets, dtype=np.float32).reshape(-1)
    assert o.shape == (L,) and t.shape == (L,)

    in_maps = _make_in_maps(o, t)
    nc = _get_nc()
    res = run_bass_kernel_spmd(nc, in_maps, core_ids=list(range(N_CORES)))

    wave_sum = 0.0
    cwt_sum = 0.0
    for c in range(N_CORES):
        p = np.asarray(res.results[c]["partials"], dtype=np.float64)
        wave_sum += p[:, 0].sum()
        cwt_sum += p[:, 1:9].sum()
    loss = ALPHA * wave_sum / L + (1.0 - ALPHA) * cwt_sum / (NW * L)
    return np.float32(loss)


# revision 29
# speedup vs baseline: 2.5434x; 1.0090x over previous
"""Trainium2 Bass kernel for nn_CombinedLoss (L1 wave + L1 on real-morlet CWT).

Math: loss = 0.5*mean|o-t| + 0.5*mean|CWT(o)-CWT(t)|.  Convolution is
linear, so CWT(o)-CWT(t) = CWT(d) with d = o-t.

Mapping: width-sharded SPMD (the 36 wavelet widths are distributed over
the 8 cores; every core convolves the full replicated signal with its
4-5 widths).  Each width's banded-Toeplitz conv runs on the tensor
engine as fp8 DoubleRow matmuls: one "unit" contracts 256 consecutive
input samples (2 k-tiles of 128) against a [128,2,128] weight block,
with per-width output shifts S_w = (5w mod 128) chosen so small widths
(1..12) need 1 unit and large widths (13..36) need 2 - 8 units/core.

The moving operand needs k-tile t of output column j to read signal
column (base+t+j); since the PE rejects overlapping-stride APs, the
host supplies the transposed difference signal twice (shift-0/shift-1
planes) so the DoubleRow view is a plain slice.  |.|-sums of the psum
banks are split between DVE (tensor_reduce) and ACT (activation Abs +
accum); per-core partials are combined on the host (the all-reduce).
"""

import numpy as np
import ml_dtypes

import concourse.bass as bass
import concourse.tile as tile
import concourse.mybir as mybir
from concourse.bass_utils import run_bass_kernel_spmd
from concourse.vector_clock import ScopedClock

L = 262144
NW = 36
ALPHA = 0.5
N_CORES = 8
NCOLS = 2048                 # output columns (128 samples each)
PAD = 4                      # zero columns left of the signal
SIGW = 2060                  # PAD + 2048 + 8
WAVE0 = SIGW                 # wave-slice columns start
D2W = SIGW + 256             # + per-core wave slice
F32 = mybir.dt.float32
BF16 = mybir.dt.bfloat16
FP8 = mybir.dt.float8e4
FP8_NP = ml_dtypes.float8_e4m3

# slot structure (identical on every core): (units, c0)
SLOT_UNITS = [2, 2, 2, 1, 1]
SLOT_C0 = [-1, -1, -1, 0, 0]
UNIT_OFF = [0, 2, 4, 6, 7]   # first unit index of each slot
N_UNITS = 8
N_PHASES = 4                 # 4 x 512 output columns
WARMUP_MM = 58

# input DMA blocks (d2 plane-pair column ranges)
D2_BLOCKS = [(0, 520), (520, 1554), (1554, D2W)]

# per-phase psum bank map (slot -> bank) and reduce plan (bank_lo, nbanks,
# engine).  All reduce groups are address-contiguous; triples alternate
# between ACT and DVE so neither engine's chain gates two phases in a row.
BANK_MAP = [
    [0, 1, 2, 3, 4],
    [5, 6, 7, 0, 1],
    [2, 3, 4, 5, 6],
    [5, 6, 7, 3, 4],
]
REDUCE_PLAN = [
    [(0, 2, "dve"), (2, 3, "act")],
    [(5, 3, "act"), (0, 2, "dve")],
    [(2, 3, "act"), (5, 2, "dve")],
    [(5, 3, "act"), (3, 2, "dve")],
]


def core_widths(c):
    """5 width slots for core c (0 = zero/padding slot)."""
    return [13 + c, 21 + c, 29 + c, 1 + c, 9 + c if c < 4 else 0]


def _morlet(N, w):
    x = np.linspace(-2.0 * np.pi, 2.0 * np.pi, N)
    return (np.cos(w * x) - np.exp(-0.5 * w * w)) * np.exp(-0.5 * x * x) * np.pi ** (-0.25)


def _build_core_weights(c):
    """[128, 8*256] fp8 weight layout for core c."""
    W = np.zeros((128, N_UNITS, 2, 128), np.float32)
    k = np.arange(128)[:, None]
    i = np.arange(128)[None, :]
    for s, w in enumerate(core_widths(c)):
        if w == 0:
            continue
        N, a0 = 10 * w, 5 * w
        S = a0 % 128
        g = _morlet(N, float(w))
        for u in range(SLOT_UNITS[s]):
            for t in range(2):
                m = k - i - S + a0 + 128 * (SLOT_C0[s] + 2 * u + t)
                W[:, UNIT_OFF[s] + u, t, :] = np.where(
                    (m >= 0) & (m < N), g[np.clip(m, 0, N - 1)], 0.0
                )
    return W.reshape(128, N_UNITS * 256).astype(FP8_NP)


_CORE_WEIGHTS = [_build_core_weights(c) for c in range(N_CORES)]


class _TC(tile.TileContext):
    """TileContext whose tail drain carries at most one sync wait (the
    walrus build in this container rejects multi-wait Drains)."""

    def _lower_ordered_insts(self, ordered):
        nc = self.nc
        for bb_name in list(ordered.keys()):
            insts = ordered[bb_name]
            new = []
            for inst in insts:
                si = inst.sync_info
                if si is not None and len(si.on_wait) > 1:
                    waits = list(si.on_wait)
                    for w in waits[:-1]:
                        nop = mybir.InstEventSemaphore(
                            name=f"wsplit-{nc.next_id()}", ins=[], outs=[],
                            engine=inst.engine,
                        )
                        nop.sync_info = mybir.SyncInfo(on_wait=[w], on_update=[])
                        nc.register_instruction(nop, overwrite=True)
                        new.append(nop)
                    inst.sync_info = mybir.SyncInfo(
                        on_wait=[waits[-1]], on_update=list(si.on_update)
                    )
                new.append(inst)
            ordered[bb_name] = new
        return super()._lower_ordered_insts(ordered)

    def _drain_and_barrier(self, tick_clock, wait_clock):
        nc = self.nc
        probe = mybir.InstDrain(
            name=f"probe-{nc.next_id()}", ins=[], outs=[], engine=mybir.EngineType.SP
        )
        wait_clock.add_sem_waits(probe, ScopedClock({None: tick_clock.global_clock}))
        si = probe.sync_info
        waits = list(si.on_wait) if si is not None else []
        allocated = self.sems.allocated()
        handles = list(allocated.values()) if isinstance(allocated, dict) else list(allocated)
        id2sem = {h.num: h for h in handles}
        name2sem = {h.name: h for h in handles}
        for w in waits:
            sem = id2sem.get(w.id) or name2sem.get(w.ant_name)
            assert sem is not None, (w.id, w.ant_name, sorted(id2sem))
            nc.sync.wait_ge(sem, w.wait_value)
        nc.sync.drain()
        nc.all_engine_barrier()
        popped = nc._tile_sem_poison_stack.pop()
        assert popped is self._sem_poison
        nc.clear_and_free_semaphores(list(self.sems.allocated().values()))
        nc.all_engine_barrier()


_NC_CACHE = None


def _build_nc():
    nc = bass.Bass("TRN2", target_bir_lowering=False, debug=False, num_devices=N_CORES)
    d2_ext = nc.dram_tensor("d2", [128, 2, D2W], FP8, kind="ExternalInput")
    w_ext = nc.dram_tensor("wts", [128, N_UNITS * 256], FP8, kind="ExternalInput")
    out_ext = nc.dram_tensor("partials", [128, 16], F32, kind="ExternalOutput")

    with _TC(nc) as tc:
        with (
            tc.tile_pool(name="sig", bufs=1) as sig_pool,
            tc.tile_pool(name="wt", bufs=1) as wt_pool,
            tc.tile_pool(name="misc", bufs=1) as misc_pool,
            tc.tile_pool(name="ps", bufs=1, space="PSUM") as ps_pool,
        ):
            d2_sb = sig_pool.tile([128, 2, D2W], FP8, tag="d2")
            w_sb = wt_pool.tile([128, N_UNITS, 2, 128], FP8, tag="w")
            scratch = misc_pool.tile([128, 1536], BF16, tag="scr")
            parts = misc_pool.tile([128, 16], F32, tag="parts")
            warm = misc_pool.tile([128, 2, 128], FP8, tag="warm")
            psum = ps_pool.tile([128, 4096], F32, tag="psum")

            # ---- input DMAs (SP engine queue, in priority order) ----
            lo, hi = D2_BLOCKS[0]
            nc.sync.dma_start(d2_sb[:, :, lo:hi], d2_ext[:, :, lo:hi])
            nc.sync.dma_start(w_sb[:, 0:4], w_ext[:, 0:1024])      # slots 0,1
            nc.sync.dma_start(w_sb[:, 4:8], w_ext[:, 1024:2048])   # slots 2,3,4
            for lo, hi in D2_BLOCKS[1:]:
                nc.sync.dma_start(d2_sb[:, :, lo:hi], d2_ext[:, :, lo:hi])

            nc.gpsimd.memset(warm[:], 0.0)
            nc.gpsimd.memset(parts[:], 0.0)

            # ---- PE warmup (p-state ramp bridge; zeros, never read) ----
            for _ in range(WARMUP_MM):
                nc.tensor.matmul(
                    psum[:, 3584:3712], warm[:], warm[:],
                    start=True, stop=True,
                    perf_mode=mybir.MatmulPerfMode.DoubleRow,
                )

            # ---- conv matmuls + reduces, phase by phase ----
            red_col = 1
            for jb in range(N_PHASES):
                for s in range(5):
                    bank = BANK_MAP[jb][s]
                    nu = SLOT_UNITS[s]
                    for u in range(nu):
                        base = PAD + SLOT_C0[s] + 2 * u + 512 * jb
                        nc.tensor.matmul(
                            psum[:, 512 * bank:512 * (bank + 1)],
                            w_sb[:, UNIT_OFF[s] + u],
                            d2_sb[:, :, base:base + 512],
                            start=(u == 0), stop=(u == nu - 1),
                            perf_mode=mybir.MatmulPerfMode.DoubleRow,
                        )
                for lo_bank, nb, eng in REDUCE_PLAN[jb]:
                    src = psum[:, 512 * lo_bank:512 * (lo_bank + nb)]
                    if eng == "dve":
                        nc.vector.tensor_reduce(
                            parts[:, red_col:red_col + 1], src,
                            axis=mybir.AxisListType.X, op=mybir.AluOpType.add,
                            apply_absolute_value=True,
                        )
                    else:
                        nc.scalar.activation(
                            scratch[:, 0:512 * nb], src,
                            mybir.ActivationFunctionType.Abs,
                            accum_out=parts[:, red_col:red_col + 1],
                        )
                    red_col += 1
                if jb == 1:
                    # wave term: |d| over this core's slice, in DVE's idle
                    # window between its ph1 and ph2 pair-reduces
                    nc.vector.tensor_reduce(
                        parts[:, 0:1], d2_sb[:, 0, WAVE0:WAVE0 + 256],
                        axis=mybir.AxisListType.X, op=mybir.AluOpType.add,
                        apply_absolute_value=True,
                    )

            nc.sync.dma_start(out_ext[:], parts[:])
    return nc


def _get_nc():
    global _NC_CACHE
    if _NC_CACHE is None:
        _NC_CACHE = _build_nc()
    return _NC_CACHE


def _make_in_maps(o, t):
    d = (o - t).astype(FP8_NP)
    dT = d.reshape(NCOLS, 128).T                      # [128, 2048]
    d2 = np.zeros((128, 2, D2W), FP8_NP)
    d2[:, 0, PAD:PAD + NCOLS] = dT
    d2[:, 1, PAD - 1:PAD - 1 + NCOLS] = dT            # shift-1 plane

    in_maps = []
    for c in range(N_CORES):
        m = d2.copy()
        m[:, 0, WAVE0:WAVE0 + 256] = dT[:, 256 * c:256 * (c + 1)]
        in_maps.append({"d2": m, "wts": _CORE_WEIGHTS[c]})
    return in_maps


def kernel(outputs, targets):
    o = np.asarray(outputs, dtype=np.float32).reshape(-1)
    t = np.asarray(targets, dtype=np.float32).reshape(-1)
    assert o.shape == (L,) and t.shape == (L,)

    in_maps = _make_in_maps(o, t)
    nc = _get_nc()
    res = run_bass_kernel_spmd(nc, in_maps, core_ids=list(range(N_CORES)))

    wave_sum = 0.0
    cwt_sum = 0.0
    for c in range(N_CORES):
        p = np.asarray(res.results[c]["partials"], dtype=np.float64)
        wave_sum += p[:, 0].sum()
        cwt_sum += p[:, 1:9].sum()
    loss = ALPHA * wave_sum / L + (1.0 - ALPHA) * cwt_sum / (NW * L)
    return np.float32(loss)


# revision 30
# speedup vs baseline: 2.5582x; 1.0058x over previous
"""Trainium2 Bass kernel for nn_CombinedLoss (L1 wave + L1 on real-morlet CWT).

Math: loss = 0.5*mean|o-t| + 0.5*mean|CWT(o)-CWT(t)|.  Convolution is
linear, so CWT(o)-CWT(t) = CWT(d) with d = o-t.

Mapping: width-sharded SPMD (the 36 wavelet widths are distributed over
the 8 cores; every core convolves the full replicated signal with its
4-5 widths).  Each width's banded-Toeplitz conv runs on the tensor
engine as fp8 DoubleRow matmuls: one "unit" contracts 256 consecutive
input samples (2 k-tiles of 128) against a [128,2,128] weight block,
with per-width output shifts S_w = (5w mod 128) chosen so small widths
(1..12) need 1 unit and large widths (13..36) need 2 - 8 units/core.

The moving operand needs k-tile t of output column j to read signal
column (base+t+j); since the PE rejects overlapping-stride APs, the
host supplies the transposed difference signal twice (shift-0/shift-1
planes) so the DoubleRow view is a plain slice.  |.|-sums of the psum
banks are split between DVE (tensor_reduce) and ACT (activation Abs +
accum); per-core partials are combined on the host (the all-reduce).
"""

import numpy as np
import ml_dtypes

import concourse.bass as bass
import concourse.tile as tile
import concourse.mybir as mybir
from concourse.bass_utils import run_bass_kernel_spmd
from concourse.vector_clock import ScopedClock

L = 262144
NW = 36
ALPHA = 0.5
N_CORES = 8
NCOLS = 2048                 # output columns (128 samples each)
PAD = 4                      # zero columns left of the signal
SIGW = 2060                  # PAD + 2048 + 8
WAVE0 = SIGW                 # wave-slice columns start
D2W = SIGW + 256             # + per-core wave slice
F32 = mybir.dt.float32
BF16 = mybir.dt.bfloat16
FP8 = mybir.dt.float8e4
FP8_NP = ml_dtypes.float8_e4m3

# slot structure (identical on every core): (units, c0)
SLOT_UNITS = [2, 2, 2, 1, 1]
SLOT_C0 = [-1, -1, -1, 0, 0]
UNIT_OFF = [0, 2, 4, 6, 7]   # first unit index of each slot
N_UNITS = 8
N_PHASES = 4                 # 4 x 512 output columns
WARMUP_MM = 58

# input DMA blocks (d2 plane-pair column ranges)
D2_BLOCKS = [(0, 520), (520, 1554), (1554, D2W)]

# per-phase psum bank map (slot -> bank) and reduce plan (bank_lo, nbanks,
# engine).  All reduce groups are address-contiguous; triples alternate
# between ACT and DVE so neither engine's chain gates two phases in a row.
BANK_MAP = [
    [0, 1, 2, 3, 4],
    [5, 6, 7, 0, 1],
    [2, 3, 4, 0, 1],
    [5, 6, 7, 0, 1],
]
REDUCE_PLAN = [
    [(0, 2, "dve"), (2, 3, "act")],
    [(5, 3, "act"), (0, 2, "dve")],
    [(2, 3, "act"), (0, 2, "dve")],
    [(5, 3, "act"), (0, 2, "dve")],
]


def core_widths(c):
    """5 width slots for core c (0 = zero/padding slot)."""
    return [13 + c, 21 + c, 29 + c, 1 + c, 9 + c if c < 4 else 0]


def _morlet(N, w):
    x = np.linspace(-2.0 * np.pi, 2.0 * np.pi, N)
    return (np.cos(w * x) - np.exp(-0.5 * w * w)) * np.exp(-0.5 * x * x) * np.pi ** (-0.25)


def _build_core_weights(c):
    """[128, 8*256] fp8 weight layout for core c."""
    W = np.zeros((128, N_UNITS, 2, 128), np.float32)
    k = np.arange(128)[:, None]
    i = np.arange(128)[None, :]
    for s, w in enumerate(core_widths(c)):
        if w == 0:
            continue
        N, a0 = 10 * w, 5 * w
        S = a0 % 128
        g = _morlet(N, float(w))
        for u in range(SLOT_UNITS[s]):
            for t in range(2):
                m = k - i - S + a0 + 128 * (SLOT_C0[s] + 2 * u + t)
                W[:, UNIT_OFF[s] + u, t, :] = np.where(
                    (m >= 0) & (m < N), g[np.clip(m, 0, N - 1)], 0.0
                )
    return W.reshape(128, N_UNITS * 256).astype(FP8_NP)


_CORE_WEIGHTS = [_build_core_weights(c) for c in range(N_CORES)]


class _TC(tile.TileContext):
    """TileContext whose tail drain carries at most one sync wait (the
    walrus build in this container rejects multi-wait Drains)."""

    def _lower_ordered_insts(self, ordered):
        nc = self.nc
        for bb_name in list(ordered.keys()):
            insts = ordered[bb_name]
            new = []
            for inst in insts:
                si = inst.sync_info
                if si is not None and len(si.on_wait) > 1:
                    waits = list(si.on_wait)
                    for w in waits[:-1]:
                        nop = mybir.InstEventSemaphore(
                            name=f"wsplit-{nc.next_id()}", ins=[], outs=[],
                            engine=inst.engine,
                        )
                        nop.sync_info = mybir.SyncInfo(on_wait=[w], on_update=[])
                        nc.register_instruction(nop, overwrite=True)
                        new.append(nop)
                    inst.sync_info = mybir.SyncInfo(
                        on_wait=[waits[-1]], on_update=list(si.on_update)
                    )
                new.append(inst)
            ordered[bb_name] = new
        return super()._lower_ordered_insts(ordered)

    def _drain_and_barrier(self, tick_clock, wait_clock):
        nc = self.nc
        probe = mybir.InstDrain(
            name=f"probe-{nc.next_id()}", ins=[], outs=[], engine=mybir.EngineType.SP
        )
        wait_clock.add_sem_waits(probe, ScopedClock({None: tick_clock.global_clock}))
        si = probe.sync_info
        waits = list(si.on_wait) if si is not None else []
        allocated = self.sems.allocated()
        handles = list(allocated.values()) if isinstance(allocated, dict) else list(allocated)
        id2sem = {h.num: h for h in handles}
        name2sem = {h.name: h for h in handles}
        for w in waits:
            sem = id2sem.get(w.id) or name2sem.get(w.ant_name)
            assert sem is not None, (w.id, w.ant_name, sorted(id2sem))
            nc.sync.wait_ge(sem, w.wait_value)
        nc.sync.drain()
        nc.all_engine_barrier()
        popped = nc._tile_sem_poison_stack.pop()
        assert popped is self._sem_poison
        nc.clear_and_free_semaphores(list(self.sems.allocated().values()))
        nc.all_engine_barrier()


_NC_CACHE = None


def _build_nc():
    nc = bass.Bass("TRN2", target_bir_lowering=False, debug=False, num_devices=N_CORES)
    d2_ext = nc.dram_tensor("d2", [128, 2, D2W], FP8, kind="ExternalInput")
    w_ext = nc.dram_tensor("wts", [128, N_UNITS * 256], FP8, kind="ExternalInput")
    out_ext = nc.dram_tensor("partials", [128, 16], F32, kind="ExternalOutput")

    with _TC(nc) as tc:
        with (
            tc.tile_pool(name="sig", bufs=1) as sig_pool,
            tc.tile_pool(name="wt", bufs=1) as wt_pool,
            tc.tile_pool(name="misc", bufs=1) as misc_pool,
            tc.tile_pool(name="ps", bufs=1, space="PSUM") as ps_pool,
        ):
            d2_sb = sig_pool.tile([128, 2, D2W], FP8, tag="d2")
            w_sb = wt_pool.tile([128, N_UNITS, 2, 128], FP8, tag="w")
            scratch = misc_pool.tile([128, 1536], BF16, tag="scr")
            parts = misc_pool.tile([128, 16], F32, tag="parts")
            warm = misc_pool.tile([128, 2, 128], FP8, tag="warm")
            psum = ps_pool.tile([128, 4096], F32, tag="psum")

            # ---- input DMAs (SP engine queue, in priority order) ----
            lo, hi = D2_BLOCKS[0]
            nc.sync.dma_start(d2_sb[:, :, lo:hi], d2_ext[:, :, lo:hi])
            nc.sync.dma_start(w_sb[:, 0:4], w_ext[:, 0:1024])      # slots 0,1
            nc.sync.dma_start(w_sb[:, 4:8], w_ext[:, 1024:2048])   # slots 2,3,4
            for lo, hi in D2_BLOCKS[1:]:
                nc.sync.dma_start(d2_sb[:, :, lo:hi], d2_ext[:, :, lo:hi])

            nc.gpsimd.memset(warm[:], 0.0)
            nc.gpsimd.memset(parts[:], 0.0)

            # ---- PE warmup (p-state ramp bridge; zeros, never read) ----
            for _ in range(WARMUP_MM):
                nc.tensor.matmul(
                    psum[:, 3584:3712], warm[:], warm[:],
                    start=True, stop=True,
                    perf_mode=mybir.MatmulPerfMode.DoubleRow,
                )

            # ---- conv matmuls + reduces, phase by phase ----
            red_col = 1
            for jb in range(N_PHASES):
                for s in range(5):
                    bank = BANK_MAP[jb][s]
                    nu = SLOT_UNITS[s]
                    for u in range(nu):
                        base = PAD + SLOT_C0[s] + 2 * u + 512 * jb
                        nc.tensor.matmul(
                            psum[:, 512 * bank:512 * (bank + 1)],
                            w_sb[:, UNIT_OFF[s] + u],
                            d2_sb[:, :, base:base + 512],
                            start=(u == 0), stop=(u == nu - 1),
                            perf_mode=mybir.MatmulPerfMode.DoubleRow,
                        )
                for lo_bank, nb, eng in REDUCE_PLAN[jb]:
                    src = psum[:, 512 * lo_bank:512 * (lo_bank + nb)]
                    if eng == "dve":
                        nc.vector.tensor_reduce(
                            parts[:, red_col:red_col + 1], src,
                            axis=mybir.AxisListType.X, op=mybir.AluOpType.add,
                            apply_absolute_value=True,
                        )
                    else:
                        nc.scalar.activation(
                            scratch[:, 0:512 * nb], src,
                            mybir.ActivationFunctionType.Abs,
                            accum_out=parts[:, red_col:red_col + 1],
                        )
                    red_col += 1
                if jb == 1:
                    # wave term: |d| over this core's slice, in DVE's idle
                    # window between its ph1 and ph2 pair-reduces
                    nc.vector.tensor_reduce(
                        parts[:, 0:1], d2_sb[:, 0, WAVE0:WAVE0 + 256],
                        axis=mybir.AxisListType.X, op=mybir.AluOpType.add,
                        apply_absolute_value=True,
                    )

            nc.sync.dma_start(out_ext[:], parts[:])
    return nc


def _get_nc():
    global _NC_CACHE
    if _NC_CACHE is None:
        _NC_CACHE = _build_nc()
    return _NC_CACHE


def _make_in_maps(o, t):
    d = (o - t).astype(FP8_NP)
    dT = d.reshape(NCOLS, 128).T                      # [128, 2048]
    d2 = np.zeros((128, 2, D2W), FP8_NP)
    d2[:, 0, PAD:PAD + NCOLS] = dT
    d2[:, 1, PAD - 1:PAD - 1 + NCOLS] = dT            # shift-1 plane

    in_maps = []
    for c in range(N_CORES):
        m = d2.copy()
        m[:, 0, WAVE0:WAVE0 + 256] = dT[:, 256 * c:256 * (c + 1)]
        in_maps.append({"d2": m, "wts": _CORE_WEIGHTS[c]})
    return in_maps


def kernel(outputs, targets):
    o = np.asarray(outputs, dtype=np.float32).reshape(-1)
    t = np.asarray(targets, dtype=np.float32).reshape(-1)
    assert o.shape == (L,) and t.shape == (L,)

    in_maps = _make_in_maps(o, t)
    nc = _get_nc()
    res = run_bass_kernel_spmd(nc, in_maps, core_ids=list(range(N_CORES)))

    wave_sum = 0.0
    cwt_sum = 0.0
    for c in range(N_CORES):
        p = np.asarray(res.results[c]["partials"], dtype=np.float64)
        wave_sum += p[:, 0].sum()
        cwt_sum += p[:, 1:9].sum()
    loss = ALPHA * wave_sum / L + (1.0 - ALPHA) * cwt_sum / (NW * L)
    return np.float32(loss)
